# revision 19
# baseline (speedup 1.0000x reference)
# Trainium2 Bass kernel for DiffeomorphicNetwork (scaling-and-squaring warp + TopoFix).
#
# Key algorithmic fact: |flow_k|_inf <= 1.5 * 2^k / 64, so every squaring step
# displaces < 1 voxel (3-tap stencil reach) and the final mask warp < 2 voxels
# (5-tap reach). Trilinear gather therefore becomes a dense separable stencil
# with per-voxel hat weights  hat(f - o) = max(0, 1 - |f - o|); clamping the
# fractional flow to the global volume bounds makes out-of-range tap weights
# exactly zero, which reproduces border (clamp) sampling semantics exactly.
#
# Sharding: data-parallel over (batch, z-quarter) across 8 cores, zero
# collectives. Each core computes its 32 output slices from a 44-slice halo
# window; halo validity shrinks by one slice per squaring step.
#
# Device layout: partition = y (128 rows), free dim = flat run of z-blocks of
# 132 columns ([2 pad | 128 x | 2 pad]). x/z shifts are free-dim offsets.
# The flow field ping-pongs between two SBUF-resident buffers (no DRAM
# round-trip); y-shifted difference fields Ey = V(y+1)-V, Gy = V(y-1)-V are
# produced by the (otherwise idle) PE array as matmuls against constant
# bidiagonal shift matrices whose edge rows encode border-clamp exactly, and
# evicted from PSUM to bf16 SBUF tiles by the scalar (ACT) engine.
# The three flow channels are stacked in one (128, 3, width) tile per variant
# so each DVE instruction processes all channels (weights broadcast stride-0).
# A fraction of the z-blocks of every step is computed on the (otherwise
# idle) GPSIMD/Pool engine, which runs the same tensor_tensor stencil chain
# concurrently with DVE on disjoint blocks.

import os

import numpy as np

# The flow field lives in two big SBUF-resident buffers that every group
# slices; the default overlap-tracker work budget (100) degrades to
# whole-tensor dependencies on them, serializing the Pool engine against
# DVE. Exhaustive checking keeps the deps precise.
os.environ["TILE_EXHAUSTIVE_MEMORY_SHARE_CHECK"] = "1"

import concourse.bacc as bacc
import concourse.mybir as mybir
import concourse.tile as tile
from concourse.bass_interp import get_hw_module

F32 = mybir.dt.float32
BF16 = mybir.dt.bfloat16
AF = mybir.ActivationFunctionType
OP = mybir.AluOpType

P = 128          # partitions = y
W = 128          # x extent
PAD = 2
BLK = W + 2 * PAD  # 132 columns per z-block
NB = 44          # flow window blocks per core (32 out + 2*6 halo)
NXM = 36         # x-mask blocks per core (32 out + 2*2 halo)
NOUT = 32
NSTEP = 6
VSCALE = 1.5 / (2.0 ** NSTEP)
EPS = 1e-4
GN = 4           # z-blocks per DVE compute group (squaring steps)
GNP = 4          # z-blocks per Pool compute group (squaring steps)
GNF = 7          # z-blocks per DVE group (final 5-tap warp)
GNFP = 7         # z-blocks per Pool group (final warp)
POOL_FRAC = 0.19 # fraction of blocks offloaded to GPSIMD per step
N_CORES = 8
FLAT = 2 + NB * BLK + 2       # flat flow row length (lead/trail pads)
XMPAD = BLK + 4               # x-mask flat lead/trail pad
XMFLAT = XMPAD + NXM * BLK + XMPAD
MMCH = 396                    # matmul moving chunk (fits one PSUM bank in f32)

B, D, H, WW = 2, 128, 128, 128


def _chunks(total, size):
    out = []
    c0 = 0
    while c0 < total:
        out.append((c0, min(size, total - c0)))
        c0 += size
    return out


class _Builder:
    _dma_rr = 0

    def _dma(self):
        # Alternate between the two physical HWDGE rings.
        self._dma_rr ^= 1
        return self.nc.sync if self._dma_rr else self.nc.scalar

    def __init__(self, taps_dtype=BF16, loop_steps=0, loop_final=0):
        # loop_steps/loop_final=R>0 wrap that phase in a hardware For_i loop
        # (R iterations) for wall-clock timing builds.
        self.loop_steps = loop_steps
        self.loop_final = loop_final
        self.DT = taps_dtype
        nc = bacc.Bacc(
            "TRN2", target_bir_lowering=False, debug=False, enable_asserts=False
        )
        self.nc = nc
        self.rawv = nc.dram_tensor("rawv", (3, P, NB * BLK), F32, kind="ExternalInput").ap()
        self.xm = nc.dram_tensor("xm", (P, NXM * BLK), F32, kind="ExternalInput").ap()
        self.rawt = nc.dram_tensor("rawt", (P, NOUT * W), F32, kind="ExternalInput").ap()
        self.zb = nc.dram_tensor("zb", (P, 2, 64), F32, kind="ExternalInput").ap()
        self.yb = nc.dram_tensor("yb", (P, 2), F32, kind="ExternalInput").ap()
        self.xb = nc.dram_tensor("xb", (P, 2, BLK), F32, kind="ExternalInput").ap()
        # stacked lhsT shift/difference matrices (see shard_inputs)
        self.swts = nc.dram_tensor("swts", (6, P, P), F32, kind="ExternalInput").ap()
        self.out = nc.dram_tensor("out", (P, NOUT * W), F32, kind="ExternalOutput").ap()
        # SBUF-resident flow field ping (A). Pong (B) lives in the steps pool.
        self.flowA = nc.alloc_sbuf_tensor("flowA", [P, 3, FLAT], self.DT).ap()

    def build(self):
        nc = self.nc
        DT = self.DT
        with tile.TileContext(nc) as tc:
            with tc.tile_pool(name="const", bufs=1) as cpool:
                self.tc = tc
                self.cpool = cpool
                for val in (2.0, -1.0, -2.0, EPS, 1.0 + EPS):
                    t = nc.alloc_sbuf_tensor(f"constx-{val}", [P, 1], F32)
                    nc.gpsimd.memset(t.ap(), val)
                    nc.const_aps.aps[(F32, val)] = t.ap()
                self.yb_s = cpool.tile([P, 2], F32, tag="yb")
                nc.sync.dma_start(self.yb_s[:], self.yb)
                self.xb_s = cpool.tile([P, 2, BLK], DT, tag="xb")
                self.xg_s = cpool.tile([P, BLK], DT, tag="xg")
                self.zlo_t, self.zhi_t = [], []
                for j in range(NB):
                    lo = cpool.tile([P, 1], F32, tag=f"zlo{j}")
                    nc.sync.dma_start(lo[:], self.zb[:, 0, j : j + 1].squeeze(1))
                    hi = cpool.tile([P, 1], F32, tag=f"zhi{j}")
                    nc.sync.dma_start(hi[:], self.zb[:, 1, j : j + 1].squeeze(1))
                    self.zlo_t.append(lo)
                    self.zhi_t.append(hi)
                self.sw = cpool.tile([P, 6, P], DT, tag="sw")
                # zero lead/trail pads of flowA
                nc.vector.memset(self.flowA[:, :, 0:2], 0.0)
                nc.vector.memset(self.flowA[:, :, FLAT - 2 : FLAT], 0.0)

                with tc.tile_pool(name="prep", bufs=1) as ppool:
                    self.bpool = ppool
                    self._prep(self.flowA)
                with tc.tile_pool(name="sbig", bufs=1) as bpool, \
                     tc.tile_pool(name="swt", bufs=1) as wpool, \
                     tc.tile_pool(name="stmp", bufs=1) as tpool, \
                     tc.tile_pool(name="spsum", bufs=4, space="PSUM") as qpool:
                    self.bpool, self.wpool, self.tpool, self.qpool = \
                        bpool, wpool, tpool, qpool
                    # flow pong buffer: lives only for the steps phase
                    flowB = bpool.tile([P, 3, FLAT], DT, tag="flowB", bufs=1)
                    nc.vector.memset(flowB[:, :, 0:2], 0.0)
                    nc.vector.memset(flowB[:, :, FLAT - 2 : FLAT], 0.0)
                    # steps never write blocks 0 / NB-1 of the pong buffer but
                    # x-shift views graze their pad columns -> keep them finite
                    nc.vector.memset(flowB[:, :, 2 : 2 + BLK], 0.0)
                    nc.vector.memset(
                        flowB[:, :, 2 + (NB - 1) * BLK : 2 + NB * BLK], 0.0
                    )
                    fA = self.flowA
                    fB = flowB[:]

                    def _all_steps():
                        bufs = [fA, fB]
                        for k in range(NSTEP):
                            fin, fout = bufs[k % 2], bufs[(k + 1) % 2]
                            lo, hi = k + 1, NB - (k + 1)
                            nblk = hi - lo
                            npool = min(2 * GNP, int(round(POOL_FRAC * nblk)))
                            # Pool owns the TOP blocks and DVE issues its
                            # groups top-down: pool step k's inputs (DVE step
                            # k-1's top group + pool k-1's own output) are then
                            # ready first, so the Pool engine runs a full step
                            # ahead, completely overlapped with DVE.
                            dve_hi = hi - npool
                            g0 = dve_hi
                            while g0 < hi:
                                gn = min(GNP, hi - g0)
                                self._step_group(fin, fout, g0, gn, nc.gpsimd)
                                g0 += gn
                            ranges = []
                            g0 = lo
                            while g0 < dve_hi:
                                gn = min(GN, dve_hi - g0)
                                ranges.append((g0, gn))
                                g0 += gn
                            for g0, gn in reversed(ranges):
                                self._step_group(fin, fout, g0, gn, nc.vector)

                    if self.loop_steps:
                        with tc.For_i(0, self.loop_steps, 1):
                            _all_steps()
                    else:
                        _all_steps()
                with tc.tile_pool(name="fbig", bufs=1) as bpool, \
                     tc.tile_pool(name="fwt", bufs=1) as wpool, \
                     tc.tile_pool(name="ftmp", bufs=1) as tpool, \
                     tc.tile_pool(name="fpsum", bufs=4, space="PSUM") as qpool:
                    self.bpool, self.wpool, self.tpool, self.qpool = \
                        bpool, wpool, tpool, qpool
                    xms = bpool.tile([P, XMFLAT], DT, tag="xms", bufs=1)
                    nc.vector.memset(xms[:, 0:XMPAD], 0.0)
                    nc.vector.memset(xms[:, XMPAD + NXM * BLK :], 0.0)
                    for g0 in range(0, NXM, 12):
                        gnx = min(12, NXM - g0)
                        t = bpool.tile([P, gnx, BLK], F32, tag="prepx", bufs=2)
                        self._dma().dma_start(
                            t[:],
                            self.xm[:, g0 * BLK : (g0 + gnx) * BLK].rearrange(
                                "p (z x) -> p z x", x=BLK
                            ),
                        )
                        nc.vector.tensor_copy(
                            xms[:, XMPAD + g0 * BLK : XMPAD + (g0 + gnx) * BLK].rearrange(
                                "p (z x) -> p z x", x=BLK
                            ),
                            t[:],
                        )
                    self.xms = xms[:]

                    def _all_final():
                        # after 6 steps flow_6 is in flowA
                        g0 = 6
                        while g0 < 38:
                            gn = min(GNF, 38 - g0)
                            self._final_group(self.flowA, g0, gn, nc.vector)
                            g0 += gn

                    if self.loop_final:
                        with tc.For_i(0, self.loop_final, 1):
                            _all_final()
                    else:
                        _all_final()
        nc.compile()
        nc.m = get_hw_module(nc.m)
        return nc

    # ---------- helpers ----------
    def _prep(self, fA):
        """flow_0 = tanh(rawv) * VSCALE -> flowA (SBUF); consts -> bf16."""
        nc = self.nc
        xb_f = self.bpool.tile([P, 2, BLK], F32, tag="xbf")
        nc.sync.dma_start(xb_f[:], self.xb)
        nc.vector.tensor_copy(self.xb_s[:], xb_f[:])
        nc.vector.tensor_scalar(
            self.xg_s[:], xb_f[:, 0, :], -1.0, 0.0, OP.mult, OP.add
        )
        sw_f = self.bpool.tile([P, 6, P], F32, tag="swf")
        nc.sync.dma_start(sw_f[:], self.swts.rearrange("s p m -> p s m"))
        nc.vector.tensor_copy(self.sw[:], sw_f[:])
        for g0 in reversed(range(0, NB, 8)):
            gn = min(8, NB - g0)
            t = self.bpool.tile([P, 3, gn, BLK], F32, tag="prepf", bufs=2)
            for c in range(3):
                self._dma().dma_start(
                    t[:, c],
                    self.rawv[c][:, g0 * BLK : (g0 + gn) * BLK].rearrange(
                        "p (z x) -> p z x", x=BLK
                    ),
                )
            t2 = self.bpool.tile([P, 3, gn, BLK], self.DT, tag="prepb", bufs=2)
            nc.scalar.activation(t2[:], t[:], AF.Tanh)
            nc.scalar.mul(
                fA[:, :, 2 + g0 * BLK : 2 + (g0 + gn) * BLK].rearrange(
                    "p c (z x) -> p c z x", x=BLK
                ),
                t2[:],
                VSCALE,
            )

    def _pe_apply(self, widx, src_ap, dst, dst_off, extw, nch=3):
        """dst[:, c, dst_off+j] = (sw[:, widx, :].T @ src[:, c, :])[:, j] via PE.

        src_ap: (P, nch, extw) AP (view); dst: (P, nch, >=dst_off+extw) tile.
        Chunked matmuls into PSUM, evicted to bf16 by the ACT engine.
        """
        nc = self.nc
        lhsT = self.sw[:, widx, :]
        for c in range(nch):
            for cs, cw in _chunks(extw, MMCH):
                ps = self.qpool.tile([P, cw], F32, tag="ps", bufs=4)
                nc.tensor.matmul(
                    ps[:], lhsT, src_ap[:, c, cs : cs + cw], start=True, stop=True
                )
                nc.scalar.copy(dst[:, c, dst_off + cs : dst_off + cs + cw], ps[:])

    def _clamp_weights(self, fx, fy, fz, gn, g0, suf="", eng=None):
        """Clamped (bf16) flow fields from (P, gw) F32/DT APs.

        Pool groups (suf="p") compute everything on gpsimd so they never
        enqueue work on the busy in-order DVE queue. The Pool ISA has no
        tensor_tensor min/max, so the x-clamp uses the exact equivalent
        clamp(x + fx, 0, W-1) - x (x integer => bit-identical in fp32)."""
        nc = self.nc
        DT = self.DT
        wb = 1 if suf else 2
        eng = nc.gpsimd if suf else nc.vector
        gw = gn * BLK
        fxc = self.wpool.tile([P, gw], DT, tag="fxc" + suf, bufs=wb)
        v3 = fxc[:].rearrange("p (z x) -> p z x", x=BLK)
        fx3 = fx.rearrange("p (z x) -> p z x", x=BLK)
        if suf:
            xg = self.xg_s[:].unsqueeze(1).broadcast_to((P, gn, BLK))
            s = self.wpool.tile([P, gw], F32, tag="fxs" + suf, bufs=1)
            s3 = s[:].rearrange("p (z x) -> p z x", x=BLK)
            eng.tensor_tensor(s3, fx3, xg, OP.add)
            eng.tensor_scalar(s[:], s[:], float(W - 1), 0.0, OP.min, OP.max)
            eng.tensor_tensor(v3, s3, xg, OP.subtract)
        else:
            xhi = self.xb_s[:, 1, :].unsqueeze(1).broadcast_to((P, gn, BLK))
            xlo = self.xb_s[:, 0, :].unsqueeze(1).broadcast_to((P, gn, BLK))
            eng.tensor_tensor(v3, fx3, xhi, OP.min)
            eng.tensor_tensor(v3, v3, xlo, OP.max)
        fyc = self.wpool.tile([P, gw], DT, tag="fyc" + suf, bufs=wb)
        eng.tensor_scalar(
            fyc[:], fy, self.yb_s[:, 1:2], self.yb_s[:, 0:1], OP.min, OP.max
        )
        fzc = self.wpool.tile([P, gw], DT, tag="fzc" + suf, bufs=wb)
        for b in range(gn):
            eng.tensor_scalar(
                fzc[:, b * BLK : (b + 1) * BLK],
                fz[:, b * BLK : (b + 1) * BLK],
                self.zhi_t[g0 + b][:],
                self.zlo_t[g0 + b][:],
                OP.min,
                OP.max,
            )
        return fxc, fyc, fzc

    def _hat3(self, fc, gw, pref, suf=""):
        """hm/h0/hp hat weights. Pool-group variant ("p" suffix) runs the
        whole computation on gpsimd; DVE variant splits ACT relus + DVE h0."""
        nc = self.nc
        DT = self.DT
        if suf:
            g = nc.gpsimd
            hp = self.wpool.tile([P, gw], DT, tag=pref + "p" + suf, bufs=1)
            g.tensor_scalar_max(hp[:], fc[:], 0.0)
            hm = self.wpool.tile([P, gw], DT, tag=pref + "m" + suf, bufs=1)
            g.tensor_scalar(hm[:], fc[:], -1.0, 0.0, OP.mult, OP.max)
            h0 = self.wpool.tile([P, gw], DT, tag=pref + "0" + suf, bufs=1)
            g.tensor_tensor(h0[:], hp[:], hm[:], OP.add)
            g.tensor_scalar(h0[:], h0[:], -1.0, 1.0, OP.mult, OP.add)
            return hm, h0, hp
        hp = self.wpool.tile([P, gw], DT, tag=pref + "p", bufs=2)
        nc.scalar.activation(hp[:], fc[:], AF.Relu)
        hm = self.wpool.tile([P, gw], DT, tag=pref + "m", bufs=2)
        nc.scalar.activation(hm[:], fc[:], AF.Relu, scale=-1.0)
        ab = self.wpool.tile([P, gw], DT, tag="hab", bufs=1)
        nc.scalar.activation(ab[:], fc[:], AF.Abs)
        h0 = self.wpool.tile([P, gw], DT, tag=pref + "0", bufs=2)
        nc.vector.tensor_scalar(h0[:], ab[:], -1.0, 1.0, OP.mult, OP.add)
        return hm, h0, hp

    def _bc3(self, wt, w0, w1):
        return wt[:, w0:w1].unsqueeze(1).broadcast_to((P, 3, w1 - w0))

    def _step_group(self, fin, fout, g0, gn, eng):
        """One group of gn output z-blocks of one squaring step on engine
        `eng` (nc.vector or nc.gpsimd). All x/z-shifted operands are plain
        offset views of the SBUF flow buffer / the PE-produced Ey/Gy tiles."""
        nc = self.nc
        DT = self.DT
        suf = "p" if eng is nc.gpsimd else ""
        pb = 1 if suf else 2
        ext = gn + 2
        extw = ext * BLK
        e0 = g0 - 1
        off = 2 + e0 * BLK
        gw = gn * BLK
        win = fin[:, :, off : off + extw]
        # Ey = V(y+1)-V, Gy = V(y-1)-V. Padded by one column each side so
        # x-shifted views stay in-bounds.
        ey = self.bpool.tile([P, 3, extw + 2], DT, tag="Ey" + suf, bufs=pb)
        gy = self.bpool.tile([P, 3, extw + 2], DT, tag="Gy" + suf, bufs=pb)
        if suf:
            # Pool groups: stay entirely off the PE/ACT pipeline (their PE
            # work would queue behind the previous step's matmuls and lag a
            # full step). y-shifted copies via the idle DMA rings, then
            # subtract on gpsimd itself.
            v1 = self.bpool.tile([P, 3, extw], DT, tag="Vyp" + suf, bufs=1)
            self._dma().dma_start(v1[0 : P - 1], win[1:P])
            nc.sync.dma_start(v1[P - 1 : P], win[P - 1 : P])
            vm = self.bpool.tile([P, 3, extw], DT, tag="Vym" + suf, bufs=1)
            self._dma().dma_start(vm[1:P], win[0 : P - 1])
            nc.sync.dma_start(vm[0:1], win[0:1])
            eng.tensor_tensor(ey[:, :, 1 : 1 + extw], v1[:], win, OP.subtract)
            eng.tensor_tensor(gy[:, :, 1 : 1 + extw], vm[:], win, OP.subtract)
        else:
            # DVE groups: PE matmuls against constant bidiagonal shift
            # matrices (edge rows encode border clamp => rows 127/0 exactly
            # zero), evicted from PSUM to bf16 by the ACT engine.
            self._pe_apply(0, win, ey, 1, extw)
            self._pe_apply(1, win, gy, 1, extw)
        eng.memset(ey[:, :, 0:1], 0.0)
        eng.memset(ey[:, :, extw + 1 : extw + 2], 0.0)
        eng.memset(gy[:, :, 0:1], 0.0)
        eng.memset(gy[:, :, extw + 1 : extw + 2], 0.0)

        def V(o):   # x/z-shifted window view of the input flow
            return fin[:, :, off + o : off + o + gw]

        def EY(o):
            return ey[:, :, 1 + o : 1 + o + gw]

        def GY(o):
            return gy[:, :, 1 + o : 1 + o + gw]

        fxc, fyc, fzc = self._clamp_weights(
            fin[:, 0, off + BLK : off + BLK + gw],
            fin[:, 1, off + BLK : off + BLK + gw],
            fin[:, 2, off + BLK : off + BLK + gw],
            gn,
            g0,
            suf,
            eng,
        )
        wb = 1 if suf else 2
        fpy = self.wpool.tile([P, gw], DT, tag="fpy" + suf, bufs=wb)
        fmy = self.wpool.tile([P, gw], DT, tag="fmy" + suf, bufs=wb)
        if suf:
            eng.tensor_scalar_max(fpy[:], fyc[:], 0.0)
            eng.tensor_scalar(fmy[:], fyc[:], -1.0, 0.0, OP.mult, OP.max)
        else:
            nc.scalar.activation(fpy[:], fyc[:], AF.Relu)
            nc.scalar.activation(fmy[:], fyc[:], AF.Relu, scale=-1.0)
        hxm, hx0, hxp = self._hat3(fxc, gw, "hx", suf)
        hzm, hz0, hzp = self._hat3(fzc, gw, "hz", suf)

        fpy_b = self._bc3(fpy, 0, gw)
        fmy_b = self._bc3(fmy, 0, gw)
        I = {}
        for oz in (-1, 0, 1):
            bo = (1 + oz) * BLK  # window-start offset of this z-tap
            T = {}
            for ox in (-1, 0, 1):
                o = bo + ox
                t = self.tpool.tile([P, 3, gw], DT, tag="T" + suf, bufs=3)
                m = self.tpool.tile([P, 3, gw], DT, tag="m" + suf, bufs=1)
                eng.tensor_tensor(m[:], fpy_b, EY(o), OP.mult)
                eng.tensor_tensor(t[:], m[:], V(o), OP.add)
                eng.tensor_tensor(m[:], fmy_b, GY(o), OP.mult)
                eng.tensor_tensor(t[:], t[:], m[:], OP.add)
                T[ox] = t
            it = self.tpool.tile([P, 3, gw], DT, tag="I" + suf, bufs=3)
            eng.tensor_tensor(it[:], self._bc3(hx0, 0, gw), T[0][:], OP.mult)
            m = self.tpool.tile([P, 3, gw], DT, tag="m" + suf, bufs=1)
            eng.tensor_tensor(m[:], self._bc3(hxm, 0, gw), T[-1][:], OP.mult)
            eng.tensor_tensor(it[:], it[:], m[:], OP.add)
            eng.tensor_tensor(m[:], self._bc3(hxp, 0, gw), T[1][:], OP.mult)
            eng.tensor_tensor(it[:], it[:], m[:], OP.add)
            I[oz] = it
        ot = self.tpool.tile([P, 3, gw], DT, tag="O" + suf, bufs=1)
        eng.tensor_tensor(ot[:], self._bc3(hzm, 0, gw), I[-1][:], OP.mult)
        eng.tensor_tensor(ot[:], ot[:], V(BLK), OP.add)
        m = self.tpool.tile([P, 3, gw], DT, tag="m" + suf, bufs=1)
        eng.tensor_tensor(m[:], self._bc3(hz0, 0, gw), I[0][:], OP.mult)
        eng.tensor_tensor(ot[:], ot[:], m[:], OP.add)
        eng.tensor_tensor(m[:], self._bc3(hzp, 0, gw), I[1][:], OP.mult)
        dst = fout[:, :, 2 + g0 * BLK : 2 + g0 * BLK + gw]
        eng.tensor_tensor(dst, ot[:], m[:], OP.add)
        d4 = dst.rearrange("p c (z x) -> p c z x", x=BLK)
        eng.memset(d4[:, :, :, 0:PAD], 0.0)
        eng.memset(d4[:, :, :, W + PAD : BLK], 0.0)

    def _hat5(self, fc, gw, pref):
        nc = self.nc
        hats = {}
        for o in (-2, -1, 0, 1, 2):
            ab = self.wpool.tile([P, gw], self.DT, tag="hab", bufs=2)
            nc.scalar.activation(ab[:], fc[:], AF.Abs, bias=float(-o))
            h = self.wpool.tile([P, gw], self.DT, tag=f"{pref}h{o}", bufs=1)
            nc.scalar.activation(h[:], ab[:], AF.Relu, scale=-1.0, bias=1.0)
            hats[o] = h
        return hats

    def _final_group(self, fin, g0, gn, eng):
        """Out blocks local z in [g0, g0+gn); 5-tap warp of x-mask + TopoFix.
        y-shifted mask variants come from PE shift-matrix matmuls; x/z shifts
        are offset views."""
        nc = self.nc
        DT = self.DT
        ext = gn + 6
        extw = ext * BLK
        e0 = g0 - 7          # xm flat block index (xm idx i <-> local z i+4)
        off = XMPAD + e0 * BLK
        gw = gn * BLK
        win1 = self.xms[:, off : off + extw].unsqueeze(1)  # (P, 1, extw)
        XVT = {}
        for i, oy in enumerate((1, -1, 2, -2)):
            t = self.bpool.tile([P, extw + 4], DT, tag=f"XV{oy}", bufs=2)
            self._pe_apply(2 + i, win1, t[:].unsqueeze(1), 2, extw, nch=1)
            XVT[oy] = t

        def XV(oy, o):
            if oy == 0:
                return self.xms[:, off + o : off + o + gw]
            return XVT[oy][:, 2 + o : 2 + o + gw]

        fxc, fyc, fzc = self._clamp_weights(
            fin[:, 0, 2 + g0 * BLK : 2 + g0 * BLK + gw],
            fin[:, 1, 2 + g0 * BLK : 2 + g0 * BLK + gw],
            fin[:, 2, 2 + g0 * BLK : 2 + g0 * BLK + gw],
            gn,
            g0,
            "",
            nc.vector,
        )
        hx = self._hat5(fxc, gw, "kx")
        hy = self._hat5(fyc, gw, "ky")
        hz = self._hat5(fzc, gw, "kz")

        I = {}
        for oz in (-2, -1, 0, 1, 2):
            bo = (3 + oz) * BLK
            T = {}
            for ox in (-2, -1, 0, 1, 2):
                o = bo + ox
                t = self.tpool.tile([P, gw], DT, tag="T", bufs=6)
                m = self.tpool.tile([P, gw], DT, tag="m", bufs=2)
                eng.tensor_tensor(t[:], hy[0][:], XV(0, o), OP.mult)
                for oy in (-2, -1, 1, 2):
                    eng.tensor_tensor(m[:], hy[oy][:], XV(oy, o), OP.mult)
                    eng.tensor_tensor(t[:], t[:], m[:], OP.add)
                T[ox] = t
            it = self.tpool.tile([P, gw], DT, tag="I", bufs=6)
            eng.tensor_tensor(it[:], hx[0][:], T[0][:], OP.mult)
            m = self.tpool.tile([P, gw], DT, tag="m", bufs=2)
            for ox in (-2, -1, 1, 2):
                eng.tensor_tensor(m[:], hx[ox][:], T[ox][:], OP.mult)
                eng.tensor_tensor(it[:], it[:], m[:], OP.add)
            I[oz] = it
        wv = self.tpool.tile([P, gw], DT, tag="O", bufs=2)
        eng.tensor_tensor(wv[:], hz[0][:], I[0][:], OP.mult)
        m = self.tpool.tile([P, gw], DT, tag="m", bufs=2)
        for oz in (-2, -1, 1, 2):
            eng.tensor_tensor(m[:], hz[oz][:], I[oz][:], OP.mult)
            eng.tensor_tensor(wv[:], wv[:], m[:], OP.add)

        # TopoFix: sigmoid(log(w+eps) - log(1-w+eps) + tanh(rt)*sigmoid(rt))
        l1 = self.tpool.tile([P, gw], F32, tag="l1", bufs=2)
        nc.scalar.activation(l1[:], wv[:], AF.Ln, bias=EPS)
        l2 = self.tpool.tile([P, gw], F32, tag="l2", bufs=2)
        nc.scalar.activation(l2[:], wv[:], AF.Ln, scale=-1.0, bias=1.0 + EPS)
        nc.vector.tensor_tensor(l1[:], l1[:], l2[:], OP.subtract)

        gwr = gn * W
        rt = self.bpool.tile([P, gn, W], F32, tag="rt", bufs=2)
        self._dma().dma_start(
            rt[:],
            self.rawt[:, (g0 - 6) * W : (g0 - 6 + gn) * W].rearrange(
                "p (z x) -> p z x", x=W
            ),
        )
        rtf = rt[:].rearrange("p z x -> p (z x)")
        sg = self.tpool.tile([P, gwr], F32, tag="sg", bufs=2)
        nc.scalar.activation(sg[:], rtf, AF.Sigmoid)
        th = self.tpool.tile([P, gwr], F32, tag="th", bufs=2)
        nc.scalar.activation(th[:], rtf, AF.Tanh)
        nc.vector.tensor_tensor(sg[:], sg[:], th[:], OP.mult)
        l1v = l1[:].rearrange("p (z x) -> p z x", x=BLK)[:, :, PAD : PAD + W]
        res = self.tpool.tile([P, gn, W], F32, tag="res", bufs=2)
        nc.vector.tensor_tensor(
            res[:], l1v, sg[:].rearrange("p (z x) -> p z x", x=W), OP.add
        )
        nc.scalar.activation(res[:], res[:], AF.Sigmoid)
        self._dma().dma_start(
            self.out[:, (g0 - 6) * W : (g0 - 6 + gn) * W].rearrange(
                "p (z x) -> p z x", x=W
            ),
            res[:],
        )


# ---------------- host side ----------------

_CACHE = {}


def _build_cached():
    if "nc" not in _CACHE:
        _CACHE["nc"] = _Builder().build()
    return _CACHE["nc"]


def _shift_mats():
    """Stacked lhsT matrices (6, 128, 128) f32 for nc.tensor.matmul
    (out = lhsT.T @ V, contraction over partitions = y):
      [0] Ey:  out[y] = V[min(y+1,127)] - V[y]   (row 127 -> 0)
      [1] Gy:  out[y] = V[max(y-1,0)]   - V[y]   (row 0   -> 0)
      [2..5] S+1, S-1, S+2, S-2: out[y] = V[clamp(y+o)] for o=1,-1,2,-2
    """
    mats = np.zeros((6, P, P), np.float32)
    y = np.arange(P)
    mats[0][np.minimum(y + 1, P - 1), y] += 1.0
    mats[0][y, y] -= 1.0
    mats[1][np.maximum(y - 1, 0), y] += 1.0
    mats[1][y, y] -= 1.0
    for i, o in enumerate((1, -1, 2, -2)):
        mats[2 + i][np.clip(y + o, 0, P - 1), y] = 1.0
    return mats


def shard_inputs(x, raw):
    """Build the 8 per-core input dicts from full inputs."""
    in_maps = []
    yb = np.zeros((P, 2), np.float32)
    yb[:, 0] = -np.arange(P, dtype=np.float32)
    yb[:, 1] = (H - 1) - np.arange(P, dtype=np.float32)
    xb = np.zeros((P, 2, BLK), np.float32)
    xs = np.arange(W, dtype=np.float32)
    xb[:, 0, PAD : PAD + W] = -xs
    xb[:, 1, PAD : PAD + W] = (W - 1) - xs
    swts = _shift_mats()
    for core in range(N_CORES):
        b, q = core // 4, core % 4
        z0 = 32 * q
        gz = np.clip(z0 - 6 + np.arange(NB), 0, D - 1)
        rawv = np.zeros((3, P, NB, BLK), np.float32)
        rawv[:, :, :, PAD : PAD + W] = raw[b, 0:3][:, gz].transpose(0, 2, 1, 3)
        rawv = rawv.reshape(3, P, NB * BLK)
        gzx = np.clip(z0 - 2 + np.arange(NXM), 0, D - 1)
        xm = np.zeros((P, NXM, BLK), np.float32)
        xm[:, :, PAD : PAD + W] = x[b, 1][gzx].transpose(1, 0, 2)
        xm = xm.reshape(P, NXM * BLK)
        rawt = np.ascontiguousarray(
            raw[b, 3, z0 : z0 + NOUT].transpose(1, 0, 2).reshape(P, NOUT * W)
        )
        zb = np.zeros((P, 2, 64), np.float32)
        zgl = z0 - 6 + np.arange(NB)
        fake = (zgl < 0) | (zgl > D - 1)
        lo = np.where(fake, 0.0, -zgl.astype(np.float64)).astype(np.float32)
        hi = np.where(fake, 0.0, (D - 1) - zgl.astype(np.float64)).astype(np.float32)
        zb[:, 0, :NB] = lo
        zb[:, 1, :NB] = hi
        in_maps.append(
            {"rawv": rawv, "xm": xm, "rawt": rawt, "zb": zb, "yb": yb,
             "xb": xb, "swts": swts}
        )
    return in_maps


def kernel(x, raw):
    x = np.asarray(x, dtype=np.float32)
    raw = np.asarray(raw, dtype=np.float32)
    nc = _build_cached()
    in_maps = shard_inputs(x, raw)
    from concourse.bass_utils import run_bass_kernel_spmd

    res = run_bass_kernel_spmd(nc, in_maps, core_ids=list(range(N_CORES)))
    out = np.empty((B, 1, D, H, WW), np.float32)
    for core in range(N_CORES):
        b, q = core // 4, core % 4
        z0 = 32 * q
        out[b, 0, z0 : z0 + NOUT] = (
            res.results[core]["out"].reshape(P, NOUT, W).transpose(1, 0, 2)
        )
    return out
